# revision 2
# baseline (speedup 1.0000x reference)
"""Trainium2 Bass kernel for nn_CustomMLPLayer_13408887898971 (topk_masking).

Computes (matching reference.py):
    scores = sum_s relu(x[0,s,:])          # [d_ff]
    idx    = top_k(scores, K)              # K = 4403
    out    = x[..., idx] @ W[:, idx].T     # [1, S, d_model]

Key identity: gathering the same top-K columns of x and W and contracting
equals a dense contraction with the non-top-K columns masked to zero:
    out = (x * m) @ W.T  with  m[j] = scores[j] >= (K-th largest score)

Strategy (8 NeuronCores, tensor-parallel over d_model):
  - host: transpose x and W to j-major (contraction on partitions),
    shard W.T by d_model columns (512 per core), x.T replicated.
    GEMM operands are converted to fp16 on host (PE runs 1 cyc/row for
    fp16 vs 4 for fp32; fp16's 10 mantissa bits keep the GEMM error
    ~1e-4 rel, far under the 2e-2 gate). Scores stay exact f32.
  - device, per core:
      phase 0: DMA the W.T shard once into a resident SBUF tile
      phase A: partial scores over this core's 256-token shard
               (two-limb exact f32 summation), fp32
      phase B: AllReduce partial scores across the 8 cores (88KB)
      phase C: exact K-th largest via radix-16 binary search on the f32
               bit pattern; then mask = scores >= threshold
      phase D: mask the resident W in place (one broadcast multiply),
               then dense GEMM streaming only x tiles:
               psum[d,s] += Wm.T[jt] @ x.T[jt,s] over 86 j-tiles
  - host: concat per-core [512, 2048] out.T shards, transpose.
"""

import numpy as np

N_CORES = 8

FULL_CFG = dict(
    dff=11008,
    s=2048,
    d=4096,
    k=4403,
    name="full",
)

# matmul operand dtype: "f32" (exact, 4 cyc/row), "f16" or "bf16" (1 cyc/row)
MM_DTYPE = "f16"

_cache = {}


def _build_program(cfg):
    """Build + compile the 8-core SPMD bass program. Returns nc."""
    from concourse import bacc, tile
    import concourse.bass as bass
    import concourse.mybir as mybir
    import concourse.bass_isa as bass_isa

    dt = mybir.dt
    Alu = mybir.AluOpType

    DFF = cfg["dff"]
    S = cfg["s"]
    D = cfg["d"]
    K = cfg["k"]
    DSH = D // N_CORES           # d_model cols per core
    SSH = S // N_CORES           # score-token shard per core
    JT = DFF // 128              # j tiles
    SCH = min(512, S)            # moving free dim per matmul
    NSCH = S // SCH              # s chunks
    DT = max(1, DSH // 128)      # d tiles per core (lhsT free dim 128)
    assert DSH % 128 == 0 or DSH < 128
    DW = min(128, DSH)           # width of a d tile

    mm = cfg.get("mm_dtype", MM_DTYPE)
    mmdt = {"f32": dt.float32, "bf16": dt.bfloat16, "f16": dt.float16}[mm]

    nc = bacc.Bacc(
        "TRN2", target_bir_lowering=False, debug=False, num_devices=N_CORES
    )

    # I/O (per-core tensors; in_maps provide per-core data)
    xs = nc.dram_tensor("xs", [DFF, SSH], dt.float32, kind="ExternalInput").ap()
    xt = nc.dram_tensor("xt", [DFF, S], mmdt, kind="ExternalInput").ap()
    wt = nc.dram_tensor("wt", [DFF, DSH], mmdt, kind="ExternalInput").ap()
    outT = nc.dram_tensor("outT", [DSH, S], dt.float32, kind="ExternalOutput").ap()

    with tile.TileContext(nc) as tc:
        with (
            tc.tile_pool(name="persist", bufs=1) as pp,
            tc.tile_pool(name="xs_p", bufs=3) as xsp,
            tc.tile_pool(name="relu_p", bufs=3) as rlp,
            tc.tile_pool(name="xt_p", bufs=4) as xtp,
            tc.tile_pool(name="out_p", bufs=3) as otp,
            tc.tile_pool(name="psum", bufs=2, space="PSUM") as psp,
            tc.tile_pool(name="dram", bufs=1, space="DRAM") as drp,
        ):
            # ---- persistent small tiles ----
            # partial holds [hsum | rsum]: integer-part sums (exact in f32)
            # and fractional-residue sums of relu(x)*1024 per j.
            partial = pp.tile([128, 2 * JT], dt.float32, tag="partial")
            scores = pp.tile([128, JT], dt.float32, tag="scores")
            mask = pp.tile([128, JT], dt.float32, tag="mask")
            maskh = pp.tile([128, JT], mmdt, tag="maskh")
            thr = pp.tile([128, 1], dt.int32, tag="thr")
            cand = pp.tile([128, 1], dt.int32, tag="cand")
            ge_scr = pp.tile([128, JT], dt.float32, tag="ge_scr")
            cnts = pp.tile([128, 15], dt.float32, tag="cnts")
            cntr = pp.tile([128, 15], dt.float32, tag="cntr")
            sel = pp.tile([128, 15], dt.float32, tag="sel")
            digf = pp.tile([128, 1], dt.float32, tag="digf")
            digi = pp.tile([128, 1], dt.int32, tag="digi")

            # resident W.T shard: block t at cols [t*DSH, (t+1)*DSH)
            wres = pp.tile([128, JT * DSH], mmdt, tag="wres")

            # ---- phase 0: load W shard once (overlaps phase A) ----
            for t in range(JT):
                nc.sync.dma_start(
                    wres[:, t * DSH : (t + 1) * DSH], wt[t * 128 : (t + 1) * 128, :]
                )

            # ---- phase A: partial scores over this core's token shard ----
            # Scores must effectively match fp64 accuracy: the reference's
            # top-K boundary gap (~4e-4 abs) is only a few f32 ULP, so a
            # plain f32 running sum (noise ~3e-4) flips boundary neurons.
            # Two-limb trick on r = relu(x)*1024: the integer part h sums
            # EXACTLY in f32 (all partials are integers < 2^24), and the
            # fractional part r1 < 1 sums with noise ~1e-6 relative.
            for t in range(JT):
                st = xsp.tile([128, SSH], dt.float32)
                nc.sync.dma_start(st[:], xs[t * 128 : (t + 1) * 128, :])
                rt = rlp.tile([128, SSH], dt.float32, tag="rt")
                nc.scalar.activation(
                    rt[:],
                    st[:],
                    mybir.ActivationFunctionType.Relu,
                    scale=1024.0,
                )
                # h = round-to-nearest-int(r) via the +2^23 trick (r < 2^13),
                # exact in f32; |r1| = |r - h| <= 0.5.
                tmpt = rlp.tile([128, SSH], dt.float32, tag="tmpt")
                nc.vector.tensor_scalar(
                    out=tmpt[:],
                    in0=rt[:],
                    scalar1=float(2.0**23),
                    scalar2=None,
                    op0=Alu.add,
                )
                ht = rlp.tile([128, SSH], dt.float32, tag="ht")
                nc.vector.tensor_scalar(
                    out=ht[:],
                    in0=tmpt[:],
                    scalar1=float(2.0**23),
                    scalar2=0.0,
                    op0=Alu.subtract,
                    op1=Alu.add,
                    accum_out=partial[:, t : t + 1],
                )
                r1t = rlp.tile([128, SSH], dt.float32, tag="r1t")
                nc.vector.tensor_tensor(
                    out=r1t[:], in0=rt[:], in1=ht[:], op=Alu.subtract
                )
                nc.vector.tensor_reduce(
                    out=partial[:, JT + t : JT + t + 1],
                    in_=r1t[:],
                    axis=mybir.AxisListType.X,
                    op=Alu.add,
                )

            # ---- phase B: AllReduce partial sums across cores ----
            cc_in = drp.tile([128, 2 * JT], dt.float32)
            cc_out = drp.tile([128, 2 * JT], dt.float32)
            nc.sync.dma_start(cc_in[:], partial[:])
            nc.gpsimd.collective_compute(
                "AllReduce",
                Alu.add,
                replica_groups=[list(range(N_CORES))],
                ins=[cc_in.opt()],
                outs=[cc_out.opt()],
            )
            hr = pp.tile([128, 2 * JT], dt.float32, tag="hr")
            nc.sync.dma_start(hr[:], cc_out[:])
            # scores = (hsum + rsum) * 2^-10  (single final rounding)
            nc.vector.tensor_tensor(
                out=scores[:], in0=hr[:, :JT], in1=hr[:, JT:], op=Alu.add
            )
            nc.vector.tensor_scalar(
                out=scores[:],
                in0=scores[:],
                scalar1=float(2.0**-10),
                scalar2=None,
                op0=Alu.mult,
            )

            # ---- phase C: K-th largest via radix-16 search on f32 bits ----
            # scores >= 0, so f32 bit patterns order like int32. Candidates are
            # built in int32 bit space, and compared in f32 space (bitcast the
            # candidate): order is identical for non-negative values, and
            # candidates that land in the inf/nan range compare as "no score
            # >= cand", matching the int compare.
            # NOTE: the DVE ALU evaluates int32 tensor ops in f32 arithmetic,
            # so bit-space increments below ULP(thr_bits ~ 2^30) = 128 are
            # rounded away. The int-bit stage therefore only resolves bits
            # 7..30 (increments are multiples of 128 -> exact in f32); the
            # low 7 bits are resolved in float space using exact ULP steps.
            nc.vector.memset(thr[:], 0)

            def count_round(make_cand, ncand, upd):
                """One radix round: count candidates, pick digit, update thr."""
                for r in range(1, ncand + 1):
                    make_cand(r)
                    nc.vector.tensor_scalar(
                        out=ge_scr[:],
                        in0=scores[:],
                        scalar1=candf[:],
                        scalar2=0.0,
                        op0=Alu.is_ge,
                        op1=Alu.add,
                        accum_out=cnts[:, r - 1 : r],
                    )
                nc.gpsimd.partition_all_reduce(
                    cntr[:, :ncand],
                    cnts[:, :ncand],
                    channels=128,
                    reduce_op=bass_isa.ReduceOp.add,
                )
                nc.vector.tensor_scalar(
                    out=sel[:, :ncand],
                    in0=cntr[:, :ncand],
                    scalar1=float(K),
                    scalar2=None,
                    op0=Alu.is_ge,
                )
                nc.vector.tensor_reduce(
                    out=digf[:],
                    in_=sel[:, :ncand],
                    axis=mybir.AxisListType.X,
                    op=Alu.add,
                )
                upd()

            candf = pp.tile([128, 1], dt.float32, tag="candf")
            thr_f = pp.tile([128, 1], dt.float32, tag="thr_f")
            ulp = pp.tile([128, 1], dt.float32, tag="ulp")
            step = pp.tile([128, 1], dt.float32, tag="step")

            # --- int-bit stage: bits 7..30, radix 16 ---
            for shift in (27, 23, 19, 15, 11, 7):

                def make_cand_int(r, shift=shift):
                    nc.vector.tensor_scalar(
                        out=cand[:],
                        in0=thr[:],
                        scalar1=r << shift,
                        scalar2=None,
                        op0=Alu.add,
                    )
                    # view the int candidate as f32 for the compare
                    nc.vector.tensor_scalar(
                        out=candf[:],
                        in0=cand[:].bitcast(dt.float32),
                        scalar1=0.0,
                        scalar2=None,
                        op0=Alu.add,
                    )

                def upd_int(shift=shift):
                    nc.vector.tensor_scalar(
                        out=digi[:],
                        in0=digf[:],
                        scalar1=float(1 << shift),
                        scalar2=None,
                        op0=Alu.mult,
                    )
                    nc.vector.tensor_tensor(
                        out=thr[:], in0=thr[:], in1=digi[:], op=Alu.add
                    )

                count_round(make_cand_int, 15, upd_int)

            # --- float stage: low 7 bits with exact ULP steps ---
            # ulp = (bitcast(thr+128) - bitcast(thr)) / 128 (exact powers of 2)
            nc.vector.tensor_scalar(
                out=cand[:], in0=thr[:], scalar1=128, scalar2=None, op0=Alu.add
            )
            nc.vector.tensor_tensor(
                out=ulp[:],
                in0=cand[:].bitcast(dt.float32),
                in1=thr[:].bitcast(dt.float32),
                op=Alu.subtract,
            )
            nc.vector.tensor_scalar(
                out=ulp[:],
                in0=ulp[:],
                scalar1=1.0 / 128.0,
                scalar2=None,
                op0=Alu.mult,
            )
            nc.vector.tensor_scalar(
                out=thr_f[:],
                in0=thr[:].bitcast(dt.float32),
                scalar1=0.0,
                scalar2=None,
                op0=Alu.add,
            )

            for mult_, ncand in ((16, 7), (1, 15)):

                def make_cand_f(r, mult_=mult_):
                    nc.vector.tensor_scalar(
                        out=step[:],
                        in0=ulp[:],
                        scalar1=float(r * mult_),
                        scalar2=None,
                        op0=Alu.mult,
                    )
                    nc.vector.tensor_tensor(
                        out=candf[:], in0=thr_f[:], in1=step[:], op=Alu.add
                    )

                def upd_f(mult_=mult_):
                    nc.vector.tensor_scalar(
                        out=digf[:],
                        in0=digf[:],
                        scalar1=float(mult_),
                        scalar2=None,
                        op0=Alu.mult,
                    )
                    nc.vector.tensor_tensor(
                        out=step[:], in0=digf[:], in1=ulp[:], op=Alu.mult
                    )
                    nc.vector.tensor_tensor(
                        out=thr_f[:], in0=thr_f[:], in1=step[:], op=Alu.add
                    )

                count_round(make_cand_f, ncand, upd_f)

            # mask[j] = scores >= thr_f  (0.0/1.0), converted to mm dtype
            nc.vector.tensor_scalar(
                out=mask[:],
                in0=scores[:],
                scalar1=thr_f[:],
                scalar2=None,
                op0=Alu.is_ge,
            )
            nc.vector.tensor_copy(out=maskh[:], in_=mask[:])

            # ---- phase D: mask W in place, then dense GEMM over x tiles ----
            wres3 = wres[:].rearrange("p (t d) -> p t d", t=JT)
            mh3 = maskh[:].unsqueeze(2).broadcast_to([128, JT, DSH])
            nc.vector.tensor_tensor(out=wres3, in0=wres3, in1=mh3, op=Alu.mult)

            for c in range(NSCH):
                psums = [
                    psp.tile([DW, SCH], dt.float32, tag=f"ps{d}", name=f"ps_c{c}_d{d}")
                    for d in range(DT)
                ]
                for t in range(JT):
                    xtile = xtp.tile([128, SCH], mmdt)
                    nc.sync.dma_start(
                        xtile[:], xt[t * 128 : (t + 1) * 128, c * SCH : (c + 1) * SCH]
                    )
                    for d in range(DT):
                        nc.tensor.matmul(
                            psums[d][:],
                            lhsT=wres[:, t * DSH + d * DW : t * DSH + (d + 1) * DW],
                            rhs=xtile[:],
                            start=(t == 0),
                            stop=(t == JT - 1),
                        )
                for d in range(DT):
                    ot = otp.tile([DW, SCH], dt.float32)
                    nc.scalar.copy(ot[:], psums[d][:])
                    nc.sync.dma_start(
                        outT[d * DW : (d + 1) * DW, c * SCH : (c + 1) * SCH], ot[:]
                    )

    nc.compile()
    return nc


def _get_program(cfg):
    key = (cfg["name"], cfg.get("mm_dtype", MM_DTYPE))
    if key not in _cache:
        _cache[key] = _build_program(cfg)
    return _cache[key]


def _stage_inputs(x, W, cfg):
    """Host-side sharding/layout. Returns per-core in_maps."""
    DFF = cfg["dff"]
    S = cfg["s"]
    D = cfg["d"]
    DSH = D // N_CORES
    SSH = S // N_CORES

    x2d = np.ascontiguousarray(np.asarray(x, dtype=np.float32).reshape(S, DFF))
    Wf = np.asarray(W, dtype=np.float32)

    xT = np.ascontiguousarray(x2d.T)          # [DFF, S]
    WT = np.ascontiguousarray(Wf.T)           # [DFF, D]

    mm = cfg.get("mm_dtype", MM_DTYPE)
    if mm == "f32":
        xT_mm = xT
        WT_mm = WT
    elif mm == "f16":
        xT_mm = xT.astype(np.float16)
        WT_mm = WT.astype(np.float16)
    else:
        import ml_dtypes

        xT_mm = xT.astype(ml_dtypes.bfloat16)
        WT_mm = WT.astype(ml_dtypes.bfloat16)

    in_maps = []
    for c in range(N_CORES):
        in_maps.append(
            {
                "xs": np.ascontiguousarray(xT[:, c * SSH : (c + 1) * SSH]),
                "xt": xT_mm,
                "wt": np.ascontiguousarray(WT_mm[:, c * DSH : (c + 1) * DSH]),
            }
        )
    return in_maps


def run_cfg(x, W, cfg, trace=False, trace_kwargs=None):
    """Run the kernel for a given cfg; returns (out, BassKernelResults)."""
    from concourse.bass_utils import run_bass_kernel_spmd

    S, D = cfg["s"], cfg["d"]
    DSH = D // N_CORES
    nc = _get_program(cfg)
    in_maps = _stage_inputs(x, W, cfg)
    res = run_bass_kernel_spmd(
        nc,
        in_maps,
        core_ids=list(range(N_CORES)),
        trace=trace,
        **(trace_kwargs or {}),
    )
    outT = np.concatenate([res.results[c]["outT"] for c in range(N_CORES)], axis=0)
    out = np.ascontiguousarray(outT.T).reshape(1, S, D).astype(np.float32)
    return out, res


def kernel(x, W):
    out, _ = run_cfg(x, W, FULL_CFG)
    return out
